# revision 21
# baseline (speedup 1.0000x reference)
"""Trainium2 Bass kernel for nn_DynamicResolutionAttention.

B=2, T=2048, C=1024, H=16 heads, head_dim=64.
  q/k/v = x @ W{q,k,v}.T + b     (per-head views)
  attn  = softmax(q k^T / sqrt(hd) * (0.5 + 0.5*resolve))
  y     = attn @ v ; out = y @ Wp.T + bp
  scale = (0.5 + 0.5*resolve)/sqrt(hd)

Sharding (8 cores): core c = (batch b=c//4, head-group hg=c%4, 4 heads each).

Per core v2 pipeline (ScalarE exp is the roofline: 131072 exp elems per
partition ~= 147us at FD=1024):
 - k-major scores S^T = K_h Q_h^T; the two heads of a pair issue as
   back-to-back 64x128 row-tiled matmuls (partitions 0-63 / 64-127) which
   the PE runs concurrently -> full-array S throughput.
 - One exp ACTIVATE per head-pair k-tile ([128,2,512], FD=1024) from a
   3-deep PSUM ring; AV matmuls (V with appended ones-column, M=65) give
   softmax denominators for free.
 - Reciprocal batched per pair ([2,512]); denominator broadcast via DRAM
   round-trip; normalized y^T sent per (chunk, pair) through 8 small
   AllGathers so only the final ~6us gather is exposed.
 - Output projection in z^T orientation (stationary Wp^T chunk, N=512),
   bias folded into the PSUM eviction; host un-transposes.
 - ScalarE queue carries ONLY activations; DMAs ride sync/gpsimd/vector.

Matmul operands bf16 (fp32 PSUM accumulation); softmax statistics fp32.
"""

import sys

for _p in ("/opt/trn_rl_repo",):
    if _p not in sys.path:
        sys.path.insert(0, _p)

import numpy as np

B, T, C, H = 2, 2048, 1024, 16
HD = C // H            # 64
NCORES = 8
HL = 4                 # heads per core
NP = HL // 2           # head pairs per core
CL = HL * HD           # 256 local channels
CIN = C // 128         # 8 contraction tiles
KT_TILES = T // 128    # 16
QC = T // 512          # 4 query chunks

_prog_cache = {}


def _build_program():
    import concourse.mybir as mybir
    import concourse.tile as tile
    from concourse import bacc

    f32 = mybir.dt.float32
    bf16 = mybir.dt.bfloat16
    AG_DT = bf16   # gather payload dtype (flip to mybir.dt.float8e4)

    nc = bacc.Bacc("TRN2", target_bir_lowering=False, debug=False,
                   num_devices=NCORES)

    # host-prearranged partition-major layouts (long contiguous DMA lines)
    xP = nc.dram_tensor("xP", [128, CIN, T], bf16, kind="ExternalInput")
    wqP = nc.dram_tensor("wqP", [128, CIN, CL], bf16, kind="ExternalInput")
    wkP = nc.dram_tensor("wkP", [128, CIN, CL], bf16, kind="ExternalInput")
    wvP = nc.dram_tensor("wvP", [128, CIN, CL], bf16, kind="ExternalInput")
    wpP = nc.dram_tensor("wpP", [128, CIN, CL], bf16, kind="ExternalInput")
    bqC = nc.dram_tensor("bqC", [128, NP], f32, kind="ExternalInput")
    bkC = nc.dram_tensor("bkC", [128, NP], f32, kind="ExternalInput")
    bv = nc.dram_tensor("bv", [1, CL], bf16, kind="ExternalInput")
    bpC = nc.dram_tensor("bpC", [128, 2], f32, kind="ExternalInput")
    rlv = nc.dram_tensor("rlv", [1, 1], f32, kind="ExternalInput")
    ones_d = nc.dram_tensor("ones_d", [1, 512], bf16, kind="ExternalInput")
    zT = nc.dram_tensor("zT", [CL, T], f32, kind="ExternalOutput")

    with tile.TileContext(nc) as tc:
        with tc.tile_pool(name="const", bufs=1) as const, \
             tc.tile_pool(name="big", bufs=1) as big, \
             tc.tile_pool(name="work", bufs=4) as work, \
             tc.tile_pool(name="ps", bufs=3, space="PSUM") as ps, \
             tc.tile_pool(name="dram", bufs=1, space="DRAM") as dram:

            # runtime softmax scale: (0.5 + 0.5*resolve) / sqrt(hd)
            st = const.tile([128, 1], f32)
            nc.sync.dma_start(st[:], rlv[:].to_broadcast((128, 1)))
            nc.vector.tensor_scalar(st[:], st[:], 0.0625, 0.0625,
                                    mybir.AluOpType.mult, mybir.AluOpType.add)

            ones128 = const.tile([1, 128], bf16)
            nc.sync.dma_start(ones128[:], ones_d[:, 0:128])

            bqC_sb = const.tile([128, NP], f32)
            bkC_sb = const.tile([128, NP], f32)
            bv_sb = const.tile([1, CL], bf16)
            bpC_sb = const.tile([128, 2], f32)
            nc.gpsimd.dma_start(bqC_sb[:], bqC[:])
            nc.gpsimd.dma_start(bkC_sb[:], bkC[:])
            nc.gpsimd.dma_start(bv_sb[:], bv[:])
            nc.gpsimd.dma_start(bpC_sb[:], bpC[:])

            # weights first (small, unblock first matmuls), then x chunk 0
            # on the scalar queue (free until the first ACTIVATE), then the
            # rest of x on sync/gpsimd.
            wq_sb = big.tile([128, CIN, CL], bf16)
            wk_sb = big.tile([128, CIN, CL], bf16)
            wv_sb = big.tile([128, CIN, CL], bf16)
            wp_sb = big.tile([128, CIN, CL], bf16)
            for w_sb, w_dram, eng in ((wk_sb, wkP, nc.sync),
                                      (wq_sb, wqP, nc.gpsimd),
                                      (wv_sb, wvP, nc.sync),
                                      (wp_sb, wpP, nc.gpsimd)):
                eng.dma_start(w_sb[:, 0:4, :], w_dram[:, 0:4, :])
                eng.dma_start(w_sb[:, 4:8, :], w_dram[:, 4:8, :])

            xs = big.tile([128, CIN, T], bf16)
            c0engs = [nc.scalar, nc.sync, nc.gpsimd]
            for ci in range(CIN):
                c0engs[ci % 3].dma_start(xs[:, ci, 0:512],
                                         xP[:, ci, 0:512])
            engs = [nc.sync, nc.gpsimd]
            n_dma = 0
            for ch in range(1, QC):
                for ci in range(CIN):
                    eng = engs[n_dma % 2]
                    n_dma += 1
                    eng.dma_start(xs[:, ci, ch * 512:(ch + 1) * 512],
                                  xP[:, ci, ch * 512:(ch + 1) * 512])

            QTp = [big.tile([128, T], bf16, name=f"QT{p}") for p in range(NP)]
            KTp = [big.tile([128, T], bf16, name=f"KT{p}") for p in range(NP)]
            Vp = [big.tile([128, KT_TILES, 2, HD + 1], bf16, name=f"V{p}")
                  for p in range(NP)]
            for p in range(NP):
                nc.vector.memset(
                    Vp[p][:, :, :, HD].rearrange("p a b -> p (a b)"), 1.0)

            # ---- projection building blocks (PSUM borrowed from ring) ----
            def qk_chunk(pair, which, ch):
                """Project one 512-token chunk of Q or K for `pair`."""
                pc = slice(pair * 128, (pair + 1) * 128)
                w_sb = wq_sb if which == "q" else wk_sb
                OUT = QTp[pair] if which == "q" else KTp[pair]
                bc = bqC_sb if which == "q" else bkC_sb
                pm = ps.tile([128, 2, 512], f32, tag="w", name="pm")
                pm = pm[:, 0, :]
                for ci in range(CIN):
                    nc.tensor.matmul(
                        pm, w_sb[:, ci, pc],
                        xs[:, ci, ch * 512:(ch + 1) * 512],
                        start=(ci == 0), stop=(ci == CIN - 1))
                dst = OUT[:, ch * 512:(ch + 1) * 512]
                if which == "q":
                    # (q + bias) * temperature
                    nc.vector.tensor_scalar(
                        dst, pm, bc[:, pair:pair + 1], st[:],
                        mybir.AluOpType.add, mybir.AluOpType.mult)
                else:
                    nc.vector.tensor_scalar_add(
                        dst, pm, bc[:, pair:pair + 1])

            def v_tile(tt):
                pv = ps.tile([128, 2, 512], f32, tag="w", name="pv")
                pv = pv[:, 0, 0:CL]
                nc.tensor.matmul(pv, ones128[:], bv_sb[:],
                                 start=True, stop=False)
                for ci in range(CIN):
                    nc.tensor.matmul(
                        pv, xs[:, ci, tt * 128:(tt + 1) * 128],
                        wv_sb[:, ci, :],
                        start=False, stop=(ci == CIN - 1))
                for p in range(NP):
                    nc.vector.tensor_copy(
                        Vp[p][:, tt, :, 0:HD],
                        pv[:, p * 128:(p + 1) * 128]
                        .rearrange("p (h d) -> p h d", h=2))

            # ---- attention chunk: 2 heads of `pair` over 512 q tokens ----
            # S^T pair issued as two concurrent 64x128 row-tiled matmuls;
            # one exp ACTIVATE per k-tile covers both heads (FD=1024);
            # AV with ones-column (M=65) accumulates y^T + denominator.
            # hooks[kt] = thunks emitted into the PE stream before that
            # k-tile's S matmuls (projection / out-proj filler work).
            def attn(qc, pair, hooks=None):
                qs = slice(qc * 512, (qc + 1) * 512)
                QT_, KT_, V_ = QTp[pair], KTp[pair], Vp[pair]
                pya = ps.tile([HD + 1, 512], f32, tag="ya", name="pya",
                              bufs=1)
                pyb = ps.tile([HD + 1, 512], f32, tag="yb", name="pyb",
                              bufs=1)
                pys = (pya, pyb)
                # SW-pipelined in 2-k-tile groups: the in-order PE stream
                # runs [S,S,S,S][AV x4 of the previous group] so (a) the exp
                # stream never waits behind AV matmuls and (b) the PE pays
                # one 64x128<->128x128 tiling-mode switch per TWO k-tiles.
                def s_pair(kt):
                    ks = slice(kt * 128, (kt + 1) * 128)
                    pss = ps.tile([128, 2, 512], f32, tag="w", name="pss")
                    for hh in range(2):
                        off = hh * HD
                        nc.tensor.matmul(
                            pss[:, hh, :],
                            KT_[off:off + HD, ks],
                            QT_[off:off + HD, qs],
                            start=True, stop=True)
                    return pss

                prev = []
                for kt2 in range(0, KT_TILES, 2):
                    for kt in (kt2, kt2 + 1):
                        for thunk in (hooks or {}).get(kt, ()):
                            thunk()
                    psa = s_pair(kt2)
                    psb = s_pair(kt2 + 1)
                    for pt0, kt0 in prev:
                        for hh in range(2):
                            nc.tensor.matmul(
                                pys[hh][:], V_[:, kt0, hh, :], pt0[:, hh, :],
                                start=(kt0 == 0), stop=False)
                    prev = []
                    for kt, pss in ((kt2, psa), (kt2 + 1, psb)):
                        pt = work.tile([128, 2, 512], bf16, tag="pt",
                                       bufs=4)
                        nc.scalar.activation(
                            pt[:], pss[:], mybir.ActivationFunctionType.Exp)
                        prev.append((pt, kt))
                for pt0, kt0 in prev:
                    for hh in range(2):
                        nc.tensor.matmul(
                            pys[hh][:], V_[:, kt0, hh, :], pt0[:, hh, :],
                            start=False, stop=(kt0 == KT_TILES - 1))
                return pya, pyb

            ag_in = [[dram.tile([128, 512], AG_DT, name=f"agi{q}_{p}")
                      for p in range(NP)] for q in range(QC)]
            ag_out = [[dram.tile([4, 128, 512], AG_DT, name=f"ago{q}_{p}")
                       for p in range(NP)] for q in range(QC)]
            rec_d = dram.tile([QC * NP * 2, 512], f32, name="rec_d")

            def normalize_gather(qc, pair, pya, pyb, last=False):
                # Evacuate y + denominators out of PSUM immediately (ya/yb
                # banks gate the next chunk's AV matmuls), then fast
                # reciprocal -> DRAM broadcast -> y^T * (1/den) -> AllGather.
                den0 = work.tile([1, 512], f32, tag="den0")
                den1 = work.tile([1, 512], f32, tag="den1")
                nc.vector.tensor_copy(den0[:], pya[HD:HD + 1, :])
                ycp0 = work.tile([HD, 512], bf16, tag="ycp0", bufs=2)
                nc.vector.tensor_copy(ycp0[:], pya[0:HD, :])
                nc.vector.tensor_copy(den1[:], pyb[HD:HD + 1, :])
                ycp1 = work.tile([HD, 512], bf16, tag="ycp1", bufs=2)
                nc.vector.tensor_copy(ycp1[:], pyb[0:HD, :])
                rec0 = work.tile([1, 512], f32, tag="rec0")
                rec1 = work.tile([1, 512], f32, tag="rec1")
                nc.vector.reciprocal_approx_fast(rec0[:], den0[:])
                nc.vector.reciprocal_approx_fast(rec1[:], den1[:])
                r0 = (qc * NP + pair) * 2
                # the final chunk's DMAs ride the (now idle) scalar queue
                # so they cannot queue behind earlier sync-queue work
                dq = nc.scalar if last else nc.sync
                dq.dma_start(rec_d[r0:r0 + 1, :], rec0[:])
                dq.dma_start(rec_d[r0 + 1:r0 + 2, :], rec1[:])
                pbs0 = work.tile([HD, 512], f32, tag="pbs0")
                pbs1 = work.tile([HD, 512], f32, tag="pbs1")
                dq.dma_start(
                    pbs0[:], rec_d[r0:r0 + 1, :].to_broadcast((HD, 512)))
                dq.dma_start(
                    pbs1[:],
                    rec_d[r0 + 1:r0 + 2, :].to_broadcast((HD, 512)))
                yt = work.tile([128, 512], AG_DT, tag="yt")
                nc.vector.tensor_mul(yt[0:HD, :], ycp0[:], pbs0[:])
                nc.vector.tensor_mul(yt[HD:128, :], ycp1[:], pbs1[:])
                dq.dma_start(ag_in[qc][pair][:], yt[:])
                nc.gpsimd.collective_compute(
                    "AllGather", mybir.AluOpType.bypass,
                    replica_groups=[[0, 1, 2, 3], [4, 5, 6, 7]],
                    ins=[ag_in[qc][pair][:].opt()],
                    outs=[ag_out[qc][pair][:].opt()])

            ysbs = [big.tile([128, CIN, 512], AG_DT, name=f"ysb{q}")
                    for q in range(QC)]

            def ysb_load(qc):
                # gathered pair-halves -> 8 contraction tiles (ci=pair*4+g)
                for pair in range(NP):
                    for g in range(4):
                        nc.sync.dma_start(ysbs[qc][:, pair * 4 + g, :],
                                          ag_out[qc][pair][g, :, :])

            ysb_bf = {}

            def ysb_upcast(qc):
                if AG_DT is bf16:
                    ysb_bf[qc] = ysbs[qc]
                    return
                t = work.tile([128, CIN, 512], bf16, tag="ysbc", bufs=2)
                for ci in range(CIN):
                    nc.vector.tensor_copy(t[:, ci, :], ysbs[qc][:, ci, :])
                ysb_bf[qc] = t

            def out_proj_half(qc, half):
                # z^T[outc, tok] = sum_ci Wp^T-chunk.T @ ysb ; bias on evict
                ysb = ysb_bf[qc]
                pz = ps.tile([128, 2, 512], f32, tag="w", name="pz")
                pz = pz[:, 0, :]
                for ci in range(CIN):
                    nc.tensor.matmul(
                        pz, wp_sb[:, ci, half * 128:(half + 1) * 128],
                        ysb[:, ci, :],
                        start=(ci == 0), stop=(ci == CIN - 1))
                zTs = work.tile([128, 512], f32, tag="zTs", bufs=2)
                nc.vector.tensor_scalar_add(
                    zTs[:], pz, bpC_sb[:, half:half + 1])
                nc.sync.dma_start(
                    zT[half * 128:(half + 1) * 128,
                       qc * 512:(qc + 1) * 512], zTs[:])

            # ================= emission schedule =================
            # Pairs-inner: (0,0),(0,1),(1,0),(1,1),... so each chunk's two
            # gathers complete early and out-projs can consume them while
            # attention still runs.  Attention starts after only K0c0 +
            # Q0c0 + V-tile0; remaining V tiles and Q/K chunks are woven
            # into the in-order PE stream via per-k-tile hooks.
            qk = qk_chunk
            qk(0, "k", 0)
            qk(0, "q", 0)
            v_tile(0)

            h00 = {kt: [lambda tt=kt + 1: v_tile(tt)]
                   for kt in range(KT_TILES - 1)}
            h00[1].append(lambda: qk(1, "k", 0))
            h00[2].append(lambda: qk(0, "k", 1))
            h00[4].append(lambda: qk(1, "k", 1))
            h00[6].append(lambda: qk(0, "k", 2))
            h00[8].append(lambda: qk(1, "k", 2))
            h00[10].append(lambda: qk(0, "k", 3))
            h00[12].append(lambda: qk(1, "k", 3))
            h00[14].append(lambda: qk(1, "q", 0))
            hooks = {
                (0, 0): h00,
                (0, 1): {2: [lambda: qk(0, "q", 1)],
                         8: [lambda: qk(1, "q", 1)]},
                (1, 0): {2: [lambda: qk(0, "q", 2)],
                         8: [lambda: qk(1, "q", 2)]},
                (2, 0): {2: [lambda: qk(0, "q", 3)],
                         8: [lambda: qk(1, "q", 3)]},
                (3, 0): {4: [lambda: (ysb_load(0), ysb_upcast(0))],
                         8: [lambda: out_proj_half(0, 0)],
                         12: [lambda: out_proj_half(0, 1)]},
                (3, 1): {4: [lambda: (ysb_load(1), ysb_upcast(1))],
                         8: [lambda: out_proj_half(1, 0)],
                         12: [lambda: out_proj_half(1, 1)]},
            }
            for qc in range(QC):
                for pair in range(NP):
                    pya, pyb = attn(qc, pair, hooks.get((qc, pair)))
                    normalize_gather(qc, pair, pya, pyb,
                                     last=(qc == QC - 1 and pair == 1))
            for qc in (2, 3):
                ysb_load(qc)
                ysb_upcast(qc)
                for half in range(2):
                    out_proj_half(qc, half)

    nc.compile()
    return nc


def _get_program():
    if "nc" not in _prog_cache:
        _prog_cache["nc"] = _build_program()
    return _prog_cache["nc"]


def _pmajor(a2d):
    """[C, N] -> [128, C//128, N] partition-major contiguous."""
    Cdim, N = a2d.shape
    return np.ascontiguousarray(
        a2d.reshape(CIN, 128, N).transpose(1, 0, 2))


def kernel(x, Wq, bq, Wk, bk, Wv, bv, Wp, bp, resolve_level):
    import ml_dtypes
    from concourse.bass_utils import run_bass_kernel_spmd

    bfl = ml_dtypes.bfloat16
    nc = _get_program()

    x = np.asarray(x, np.float32)
    rl = np.asarray(resolve_level, np.float32).reshape(1, 1)

    # gathered-channel permutation: ysb ci-block (pair*4+g) holds original
    # channels [g*256 + pair*128 + 0:128] of the batch's 1024 channels
    perm = np.empty(C, np.int64)
    for pair in range(NP):
        for g in range(4):
            ci = pair * 4 + g
            perm[ci * 128:(ci + 1) * 128] = g * 256 + pair * 128 \
                + np.arange(128)

    xP_b = [_pmajor(np.ascontiguousarray(x[b].T).astype(bfl))
            for b in range(B)]
    in_maps = []
    for c in range(NCORES):
        b, hg = c // 4, c % 4
        cs = slice(hg * CL, (hg + 1) * CL)
        WpT = np.asarray(Wp, np.float32)[cs, :].T  # [C, CL]
        in_maps.append({
            "xP": xP_b[b],
            "wqP": _pmajor(np.asarray(Wq, np.float32)[cs, :].T.astype(bfl)),
            "wkP": _pmajor(np.asarray(Wk, np.float32)[cs, :].T.astype(bfl)),
            "wvP": _pmajor(np.asarray(Wv, np.float32)[cs, :].T.astype(bfl)),
            "wpP": _pmajor(np.ascontiguousarray(WpT[perm, :]).astype(bfl)),
            "bqC": np.ascontiguousarray(
                np.asarray(bq, np.float32)[cs].reshape(NP, 128).T),
            "bkC": np.ascontiguousarray(
                np.asarray(bk, np.float32)[cs].reshape(NP, 128).T),
            "bv": np.asarray(bv, np.float32)[cs].reshape(1, CL).astype(bfl),
            "bpC": np.ascontiguousarray(
                np.asarray(bp, np.float32)[cs].reshape(2, 128).T),
            "rlv": rl,
            "ones_d": np.ones((1, 512), bfl),
        })

    # First execution after a cold NEFF load can return garbage for the
    # second replica group (cross-core comm channels finish establishing
    # mid-run), so warm up once and return the second run's results.
    from concourse import bass2jax
    bass2jax.run_bass_via_pjrt(nc, in_maps, n_cores=NCORES)
    res = run_bass_kernel_spmd(nc, in_maps, core_ids=list(range(NCORES)))

    out = np.empty((B, T, C), np.float32)
    for c in range(NCORES):
        b, hg = c // 4, c % 4
        out[b, :, hg * CL:(hg + 1) * CL] = res.results[c]["zT"].T
    return out


# revision 22
# speedup vs baseline: 1.0064x; 1.0064x over previous
"""Trainium2 Bass kernel for nn_DynamicResolutionAttention.

B=2, T=2048, C=1024, H=16 heads, head_dim=64.
  q/k/v = x @ W{q,k,v}.T + b     (per-head views)
  attn  = softmax(q k^T / sqrt(hd) * (0.5 + 0.5*resolve))
  y     = attn @ v ; out = y @ Wp.T + bp
  scale = (0.5 + 0.5*resolve)/sqrt(hd)

Sharding (8 cores): core c = (batch b=c//4, head-group hg=c%4, 4 heads each).

Per core v2 pipeline (ScalarE exp is the roofline: 131072 exp elems per
partition ~= 147us at FD=1024):
 - k-major scores S^T = K_h Q_h^T; the two heads of a pair issue as
   back-to-back 64x128 row-tiled matmuls (partitions 0-63 / 64-127) which
   the PE runs concurrently -> full-array S throughput.
 - One exp ACTIVATE per head-pair k-tile ([128,2,512], FD=1024) from a
   3-deep PSUM ring; AV matmuls (V with appended ones-column, M=65) give
   softmax denominators for free.
 - Reciprocal batched per pair ([2,512]); denominator broadcast via DRAM
   round-trip; normalized y^T sent per (chunk, pair) through 8 small
   AllGathers so only the final ~6us gather is exposed.
 - Output projection in z^T orientation (stationary Wp^T chunk, N=512),
   bias folded into the PSUM eviction; host un-transposes.
 - ScalarE queue carries ONLY activations; DMAs ride sync/gpsimd/vector.

Matmul operands bf16 (fp32 PSUM accumulation); softmax statistics fp32.
"""

import sys

for _p in ("/opt/trn_rl_repo",):
    if _p not in sys.path:
        sys.path.insert(0, _p)

import numpy as np

B, T, C, H = 2, 2048, 1024, 16
HD = C // H            # 64
NCORES = 8
HL = 4                 # heads per core
NP = HL // 2           # head pairs per core
CL = HL * HD           # 256 local channels
CIN = C // 128         # 8 contraction tiles
KT_TILES = T // 128    # 16
QC = T // 512          # 4 query chunks

_prog_cache = {}


def _build_program():
    import concourse.mybir as mybir
    import concourse.tile as tile
    from concourse import bacc

    f32 = mybir.dt.float32
    bf16 = mybir.dt.bfloat16
    AG_DT = bf16   # gather payload dtype (flip to mybir.dt.float8e4)

    nc = bacc.Bacc("TRN2", target_bir_lowering=False, debug=False,
                   num_devices=NCORES)

    # host-prearranged partition-major layouts (long contiguous DMA lines)
    xP = nc.dram_tensor("xP", [128, CIN, T], bf16, kind="ExternalInput")
    wqP = nc.dram_tensor("wqP", [128, CIN, CL], bf16, kind="ExternalInput")
    wkP = nc.dram_tensor("wkP", [128, CIN, CL], bf16, kind="ExternalInput")
    wvP = nc.dram_tensor("wvP", [128, CIN, CL], bf16, kind="ExternalInput")
    wpP = nc.dram_tensor("wpP", [128, CIN, CL], bf16, kind="ExternalInput")
    bqC = nc.dram_tensor("bqC", [128, NP], f32, kind="ExternalInput")
    bkC = nc.dram_tensor("bkC", [128, NP], f32, kind="ExternalInput")
    bv = nc.dram_tensor("bv", [1, CL], bf16, kind="ExternalInput")
    bpC = nc.dram_tensor("bpC", [128, 2], f32, kind="ExternalInput")
    rlv = nc.dram_tensor("rlv", [1, 1], f32, kind="ExternalInput")
    ones_d = nc.dram_tensor("ones_d", [1, 512], bf16, kind="ExternalInput")
    zT = nc.dram_tensor("zT", [CL, T], f32, kind="ExternalOutput")

    with tile.TileContext(nc) as tc:
        with tc.tile_pool(name="const", bufs=1) as const, \
             tc.tile_pool(name="big", bufs=1) as big, \
             tc.tile_pool(name="work", bufs=4) as work, \
             tc.tile_pool(name="ps", bufs=3, space="PSUM") as ps, \
             tc.tile_pool(name="dram", bufs=1, space="DRAM") as dram:

            # runtime softmax scale: (0.5 + 0.5*resolve) / sqrt(hd)
            st = const.tile([128, 1], f32)
            nc.sync.dma_start(st[:], rlv[:].to_broadcast((128, 1)))
            nc.vector.tensor_scalar(st[:], st[:], 0.0625, 0.0625,
                                    mybir.AluOpType.mult, mybir.AluOpType.add)

            ones128 = const.tile([1, 128], bf16)
            nc.sync.dma_start(ones128[:], ones_d[:, 0:128])

            bqC_sb = const.tile([128, NP], f32)
            bkC_sb = const.tile([128, NP], f32)
            bv_sb = const.tile([1, CL], bf16)
            bpC_sb = const.tile([128, 2], f32)
            nc.gpsimd.dma_start(bqC_sb[:], bqC[:])
            nc.gpsimd.dma_start(bkC_sb[:], bkC[:])
            nc.gpsimd.dma_start(bv_sb[:], bv[:])
            nc.gpsimd.dma_start(bpC_sb[:], bpC[:])

            # weights first (small, unblock first matmuls), then x chunk 0
            # on the scalar queue (free until the first ACTIVATE), then the
            # rest of x on sync/gpsimd.
            wq_sb = big.tile([128, CIN, CL], bf16)
            wk_sb = big.tile([128, CIN, CL], bf16)
            wv_sb = big.tile([128, CIN, CL], bf16)
            wp_sb = big.tile([128, CIN, CL], bf16)
            for w_sb, w_dram, eng in ((wk_sb, wkP, nc.sync),
                                      (wq_sb, wqP, nc.gpsimd),
                                      (wv_sb, wvP, nc.sync),
                                      (wp_sb, wpP, nc.gpsimd)):
                eng.dma_start(w_sb[:, 0:4, :], w_dram[:, 0:4, :])
                eng.dma_start(w_sb[:, 4:8, :], w_dram[:, 4:8, :])

            xs = big.tile([128, CIN, T], bf16)
            c0engs = [nc.scalar, nc.sync, nc.gpsimd]
            for ci in range(CIN):
                c0engs[ci % 3].dma_start(xs[:, ci, 0:512],
                                         xP[:, ci, 0:512])
            engs = [nc.sync, nc.gpsimd]
            n_dma = 0
            for ch in range(1, QC):
                for ci in range(CIN):
                    eng = engs[n_dma % 2]
                    n_dma += 1
                    eng.dma_start(xs[:, ci, ch * 512:(ch + 1) * 512],
                                  xP[:, ci, ch * 512:(ch + 1) * 512])

            QTp = [big.tile([128, T], bf16, name=f"QT{p}") for p in range(NP)]
            KTp = [big.tile([128, T], bf16, name=f"KT{p}") for p in range(NP)]
            Vp = [big.tile([128, KT_TILES, 2, HD + 1], bf16, name=f"V{p}")
                  for p in range(NP)]
            for p in range(NP):
                nc.vector.memset(
                    Vp[p][:, :, :, HD].rearrange("p a b -> p (a b)"), 1.0)

            # ---- projection building blocks (PSUM borrowed from ring) ----
            def qk_chunk(pair, which, ch):
                """Project one 512-token chunk of Q or K for `pair`."""
                pc = slice(pair * 128, (pair + 1) * 128)
                w_sb = wq_sb if which == "q" else wk_sb
                OUT = QTp[pair] if which == "q" else KTp[pair]
                bc = bqC_sb if which == "q" else bkC_sb
                pm = ps.tile([128, 2, 512], f32, tag="w", name="pm")
                pm = pm[:, 0, :]
                for ci in range(CIN):
                    nc.tensor.matmul(
                        pm, w_sb[:, ci, pc],
                        xs[:, ci, ch * 512:(ch + 1) * 512],
                        start=(ci == 0), stop=(ci == CIN - 1))
                dst = OUT[:, ch * 512:(ch + 1) * 512]
                if which == "q":
                    # (q + bias) * temperature
                    nc.vector.tensor_scalar(
                        dst, pm, bc[:, pair:pair + 1], st[:],
                        mybir.AluOpType.add, mybir.AluOpType.mult)
                else:
                    nc.vector.tensor_scalar_add(
                        dst, pm, bc[:, pair:pair + 1])

            def v_tile(tt):
                pv = ps.tile([128, 2, 512], f32, tag="w", name="pv")
                pv = pv[:, 0, 0:CL]
                nc.tensor.matmul(pv, ones128[:], bv_sb[:],
                                 start=True, stop=False)
                for ci in range(CIN):
                    nc.tensor.matmul(
                        pv, xs[:, ci, tt * 128:(tt + 1) * 128],
                        wv_sb[:, ci, :],
                        start=False, stop=(ci == CIN - 1))
                for p in range(NP):
                    nc.vector.tensor_copy(
                        Vp[p][:, tt, :, 0:HD],
                        pv[:, p * 128:(p + 1) * 128]
                        .rearrange("p (h d) -> p h d", h=2))

            # ---- attention chunk: 2 heads of `pair` over 512 q tokens ----
            # S^T pair issued as two concurrent 64x128 row-tiled matmuls;
            # one exp ACTIVATE per k-tile covers both heads (FD=1024);
            # AV with ones-column (M=65) accumulates y^T + denominator.
            # hooks[kt] = thunks emitted into the PE stream before that
            # k-tile's S matmuls (projection / out-proj filler work).
            def attn(qc, pair, hooks=None):
                qs = slice(qc * 512, (qc + 1) * 512)
                QT_, KT_, V_ = QTp[pair], KTp[pair], Vp[pair]
                pya = ps.tile([HD + 1, 512], f32, tag="ya", name="pya",
                              bufs=1)
                pyb = ps.tile([HD + 1, 512], f32, tag="yb", name="pyb",
                              bufs=1)
                pys = (pya, pyb)
                # SW-pipelined: AV(kt-1) is emitted AFTER S(kt) so the
                # in-order PE stream never makes ACT(kt+1) wait on AV(kt);
                # the exp stream stays saturated.
                prev = None
                for kt in range(KT_TILES):
                    for thunk in (hooks or {}).get(kt, ()):
                        thunk()
                    ks = slice(kt * 128, (kt + 1) * 128)
                    pss = ps.tile([128, 2, 512], f32, tag="w", name="pss")
                    for hh in range(2):
                        off = hh * HD
                        nc.tensor.matmul(
                            pss[:, hh, :],
                            KT_[off:off + HD, ks],
                            QT_[off:off + HD, qs],
                            start=True, stop=True)
                    if prev is not None:
                        pt0, kt0 = prev
                        for hh in range(2):
                            nc.tensor.matmul(
                                pys[hh][:], V_[:, kt0, hh, :], pt0[:, hh, :],
                                start=(kt0 == 0), stop=False)
                    pt = work.tile([128, 2, 512], bf16, tag="pt", bufs=4)
                    nc.scalar.activation(
                        pt[:], pss[:], mybir.ActivationFunctionType.Exp)
                    prev = (pt, kt)
                pt0, kt0 = prev
                for hh in range(2):
                    nc.tensor.matmul(
                        pys[hh][:], V_[:, kt0, hh, :], pt0[:, hh, :],
                        start=False, stop=True)
                return pya, pyb

            ag_in = [[dram.tile([128, 512], AG_DT, name=f"agi{q}_{p}")
                      for p in range(NP)] for q in range(QC)]
            ag_out = [[dram.tile([4, 128, 512], AG_DT, name=f"ago{q}_{p}")
                       for p in range(NP)] for q in range(QC)]
            rec_d = dram.tile([QC * NP * 2, 512], f32, name="rec_d")

            def normalize_gather(qc, pair, pya, pyb, last=False):
                # Evacuate y + denominators out of PSUM immediately (ya/yb
                # banks gate the next chunk's AV matmuls), then fast
                # reciprocal -> DRAM broadcast -> y^T * (1/den) -> AllGather.
                den0 = work.tile([1, 512], f32, tag="den0")
                den1 = work.tile([1, 512], f32, tag="den1")
                nc.vector.tensor_copy(den0[:], pya[HD:HD + 1, :])
                ycp0 = work.tile([HD, 512], bf16, tag="ycp0", bufs=2)
                nc.vector.tensor_copy(ycp0[:], pya[0:HD, :])
                nc.vector.tensor_copy(den1[:], pyb[HD:HD + 1, :])
                ycp1 = work.tile([HD, 512], bf16, tag="ycp1", bufs=2)
                nc.vector.tensor_copy(ycp1[:], pyb[0:HD, :])
                rec0 = work.tile([1, 512], f32, tag="rec0")
                rec1 = work.tile([1, 512], f32, tag="rec1")
                nc.vector.reciprocal_approx_fast(rec0[:], den0[:])
                nc.vector.reciprocal_approx_fast(rec1[:], den1[:])
                r0 = (qc * NP + pair) * 2
                # the final chunk's DMAs ride the (now idle) scalar queue
                # so they cannot queue behind earlier sync-queue work
                dq = nc.scalar if last else nc.sync
                dq.dma_start(rec_d[r0:r0 + 1, :], rec0[:])
                dq.dma_start(rec_d[r0 + 1:r0 + 2, :], rec1[:])
                pbs0 = work.tile([HD, 512], f32, tag="pbs0")
                pbs1 = work.tile([HD, 512], f32, tag="pbs1")
                dq.dma_start(
                    pbs0[:], rec_d[r0:r0 + 1, :].to_broadcast((HD, 512)))
                dq.dma_start(
                    pbs1[:],
                    rec_d[r0 + 1:r0 + 2, :].to_broadcast((HD, 512)))
                yt = work.tile([128, 512], AG_DT, tag="yt")
                nc.vector.tensor_mul(yt[0:HD, :], ycp0[:], pbs0[:])
                nc.vector.tensor_mul(yt[HD:128, :], ycp1[:], pbs1[:])
                dq.dma_start(ag_in[qc][pair][:], yt[:])
                nc.gpsimd.collective_compute(
                    "AllGather", mybir.AluOpType.bypass,
                    replica_groups=[[0, 1, 2, 3], [4, 5, 6, 7]],
                    ins=[ag_in[qc][pair][:].opt()],
                    outs=[ag_out[qc][pair][:].opt()])

            ysbs = [big.tile([128, CIN, 512], AG_DT, name=f"ysb{q}")
                    for q in range(QC)]

            def ysb_load(qc):
                # gathered pair-halves -> 8 contraction tiles (ci=pair*4+g)
                for pair in range(NP):
                    for g in range(4):
                        nc.sync.dma_start(ysbs[qc][:, pair * 4 + g, :],
                                          ag_out[qc][pair][g, :, :])

            ysb_bf = {}

            def ysb_upcast(qc):
                if AG_DT is bf16:
                    ysb_bf[qc] = ysbs[qc]
                    return
                t = work.tile([128, CIN, 512], bf16, tag="ysbc", bufs=2)
                for ci in range(CIN):
                    nc.vector.tensor_copy(t[:, ci, :], ysbs[qc][:, ci, :])
                ysb_bf[qc] = t

            def out_proj_half(qc, half):
                # z^T[outc, tok] = sum_ci Wp^T-chunk.T @ ysb ; bias on evict
                ysb = ysb_bf[qc]
                pz = ps.tile([128, 2, 512], f32, tag="w", name="pz")
                pz = pz[:, 0, :]
                for ci in range(CIN):
                    nc.tensor.matmul(
                        pz, wp_sb[:, ci, half * 128:(half + 1) * 128],
                        ysb[:, ci, :],
                        start=(ci == 0), stop=(ci == CIN - 1))
                zTs = work.tile([128, 512], f32, tag="zTs", bufs=2)
                nc.vector.tensor_scalar_add(
                    zTs[:], pz, bpC_sb[:, half:half + 1])
                nc.sync.dma_start(
                    zT[half * 128:(half + 1) * 128,
                       qc * 512:(qc + 1) * 512], zTs[:])

            # ================= emission schedule =================
            # Pairs-inner: (0,0),(0,1),(1,0),(1,1),... so each chunk's two
            # gathers complete early and out-projs can consume them while
            # attention still runs.  Attention starts after only K0c0 +
            # Q0c0 + V-tile0; remaining V tiles and Q/K chunks are woven
            # into the in-order PE stream via per-k-tile hooks.
            qk = qk_chunk
            qk(0, "k", 0)
            qk(0, "q", 0)
            v_tile(0)

            h00 = {kt: [lambda tt=kt + 1: v_tile(tt)]
                   for kt in range(KT_TILES - 1)}
            h00[1].append(lambda: qk(1, "k", 0))
            h00[2].append(lambda: qk(0, "k", 1))
            h00[4].append(lambda: qk(1, "k", 1))
            h00[6].append(lambda: qk(0, "k", 2))
            h00[8].append(lambda: qk(1, "k", 2))
            h00[10].append(lambda: qk(0, "k", 3))
            h00[12].append(lambda: qk(1, "k", 3))
            h00[14].append(lambda: qk(1, "q", 0))
            hooks = {
                (0, 0): h00,
                (0, 1): {2: [lambda: qk(0, "q", 1)],
                         8: [lambda: qk(1, "q", 1)]},
                (1, 0): {2: [lambda: qk(0, "q", 2)],
                         8: [lambda: qk(1, "q", 2)]},
                (2, 0): {2: [lambda: qk(0, "q", 3)],
                         8: [lambda: qk(1, "q", 3)]},
                (3, 0): {4: [lambda: (ysb_load(0), ysb_upcast(0))],
                         8: [lambda: out_proj_half(0, 0)],
                         12: [lambda: out_proj_half(0, 1)]},
                (3, 1): {4: [lambda: (ysb_load(1), ysb_upcast(1))],
                         8: [lambda: out_proj_half(1, 0)],
                         12: [lambda: out_proj_half(1, 1)]},
            }
            for qc in range(QC):
                for pair in range(NP):
                    pya, pyb = attn(qc, pair, hooks.get((qc, pair)))
                    normalize_gather(qc, pair, pya, pyb,
                                     last=(qc == QC - 1 and pair == 1))
            for qc in (2, 3):
                ysb_load(qc)
                ysb_upcast(qc)
                for half in range(2):
                    out_proj_half(qc, half)

    nc.compile()
    return nc


def _get_program():
    if "nc" not in _prog_cache:
        _prog_cache["nc"] = _build_program()
    return _prog_cache["nc"]


def _pmajor(a2d):
    """[C, N] -> [128, C//128, N] partition-major contiguous."""
    Cdim, N = a2d.shape
    return np.ascontiguousarray(
        a2d.reshape(CIN, 128, N).transpose(1, 0, 2))


def kernel(x, Wq, bq, Wk, bk, Wv, bv, Wp, bp, resolve_level):
    import ml_dtypes
    from concourse.bass_utils import run_bass_kernel_spmd

    bfl = ml_dtypes.bfloat16
    nc = _get_program()

    x = np.asarray(x, np.float32)
    rl = np.asarray(resolve_level, np.float32).reshape(1, 1)

    # gathered-channel permutation: ysb ci-block (pair*4+g) holds original
    # channels [g*256 + pair*128 + 0:128] of the batch's 1024 channels
    perm = np.empty(C, np.int64)
    for pair in range(NP):
        for g in range(4):
            ci = pair * 4 + g
            perm[ci * 128:(ci + 1) * 128] = g * 256 + pair * 128 \
                + np.arange(128)

    xP_b = [_pmajor(np.ascontiguousarray(x[b].T).astype(bfl))
            for b in range(B)]
    in_maps = []
    for c in range(NCORES):
        b, hg = c // 4, c % 4
        cs = slice(hg * CL, (hg + 1) * CL)
        WpT = np.asarray(Wp, np.float32)[cs, :].T  # [C, CL]
        in_maps.append({
            "xP": xP_b[b],
            "wqP": _pmajor(np.asarray(Wq, np.float32)[cs, :].T.astype(bfl)),
            "wkP": _pmajor(np.asarray(Wk, np.float32)[cs, :].T.astype(bfl)),
            "wvP": _pmajor(np.asarray(Wv, np.float32)[cs, :].T.astype(bfl)),
            "wpP": _pmajor(np.ascontiguousarray(WpT[perm, :]).astype(bfl)),
            "bqC": np.ascontiguousarray(
                np.asarray(bq, np.float32)[cs].reshape(NP, 128).T),
            "bkC": np.ascontiguousarray(
                np.asarray(bk, np.float32)[cs].reshape(NP, 128).T),
            "bv": np.asarray(bv, np.float32)[cs].reshape(1, CL).astype(bfl),
            "bpC": np.ascontiguousarray(
                np.asarray(bp, np.float32)[cs].reshape(2, 128).T),
            "rlv": rl,
            "ones_d": np.ones((1, 512), bfl),
        })

    # First execution after a cold NEFF load can return garbage for the
    # second replica group (cross-core comm channels finish establishing
    # mid-run), so warm up once and return the second run's results.
    from concourse import bass2jax
    bass2jax.run_bass_via_pjrt(nc, in_maps, n_cores=NCORES)
    res = run_bass_kernel_spmd(nc, in_maps, core_ids=list(range(NCORES)))

    out = np.empty((B, T, C), np.float32)
    for c in range(NCORES):
        b, hg = c // 4, c % 4
        out[b, :, hg * CL:(hg + 1) * CL] = res.results[c]["zT"].T
    return out
